# revision 34
# baseline (speedup 1.0000x reference)
"""CaNetConv (GCN conv + gated multi-head linear) Trainium2 kernel.

Strategy (pull-mode graph SpMM, destinations sharded across 8 cores):
  hi[c,:] = sum_{e: col[e]=c} val[e] * x[row[e],:]      (GCN aggregation)
  out     = x + sum_k e[:,k] * (concat(hi,x) @ W[k])    (gated einsum)

Per core:
  - edges sorted by destination; 128-edge groups; x rows fetched with
    gpsimd dma_gather in bf16 (256B tokens)
  - DVE builds S[e,c] = (iota==colrel[e]) * val[e] in one tensor_scalar
    (all-bf16 operands -> 2x DVE rate)
  - PE computes hiT[f,c] += msg[e,f]^T @ S[e,c] (segment sum as matmul,
    bf16 operands, f32 PSUM accumulation per 128-dest block)
  - einsum as 4 bf16 matmuls per 128-node block vs preflattened W, gating
    sum via scalar_tensor_tensor with per-partition e scalars (f32), plus
    f32 residual.

One NEFF shared by all 8 cores (SPMD): the static structure (gather run
lengths, groups per block) is padded to the max across cores.

Numerics: x, val, W are rounded to bf16; all accumulations (PSUM segment
sum, PSUM einsum, gating sum, residual) stay f32. Expected rel err vs the
f32 reference ~3e-3, well under the 2e-2 gate.
"""

import sys

import numpy as np

for _p in ("/opt/trn_rl_repo", "/root/.axon_site/_ro/trn_rl_repo"):
    if _p not in sys.path:
        sys.path.append(_p)

import concourse.bass as bass  # noqa: E402
import concourse.tile as tile  # noqa: E402
from concourse import bacc, mybir  # noqa: E402

F32 = mybir.dt.float32
BF16 = mybir.dt.bfloat16
FP8 = mybir.dt.float8e4
I16 = mybir.dt.int16

import os as _osm
SUBRUN = int(_osm.environ.get("KERNEL_SUBRUN", "1024"))
_ = "(doc)"  # gather tokens per dma_gather; the SWDGE descriptor
                       # carveout (16KB/partition / 16B per desc) caps one
                       # gather at ~1024 descriptors — 1280 kills the device
SB_BLOCKS = 4          # dest blocks (128 dests) per gather superblock
PAD_COLREL = 200.0     # colrel sentinel that never matches iota 0..127


def _wrap16(a):
    """dma_gather index layout: [128, n/16], idx t at [t%16 (+16g), t//16]."""
    n = a.shape[0]
    assert n % 16 == 0
    w = a.reshape(n // 16, 16).T.astype(np.int16)  # [16, n/16]
    return np.tile(w, (8, 1))                      # replicated per Q7 core


def _prep(x, adj, e, weights, n_cores):
    """Host-side graph preprocessing. Returns (meta, in_maps)."""
    N, F = x.shape
    K = e.shape[1]
    E = adj.shape[1]
    row = np.asarray(adj[0], dtype=np.int64)
    col = np.asarray(adj[1], dtype=np.int64)

    NPC = N // n_cores                      # dests per core
    NB = (NPC + 127) // 128                 # 128-dest blocks per core
    NPCP = NB * 128
    NSB = (NB + SB_BLOCKS - 1) // SB_BLOCKS
    sb_nblocks = [min(SB_BLOCKS, NB - s * SB_BLOCKS) for s in range(NSB)]
    import os as _os
    n_half = 2 if (N > 32767 or _os.environ.get("KERNEL_FORCE_HALVES")) else 1
    HALF = (N + 1) // 2 if n_half == 2 else N

    # GCN normalization (destination degree), f32 like the reference
    deg = np.bincount(col, minlength=N).astype(np.float32)
    with np.errstate(divide="ignore"):
        r = 1.0 / np.sqrt(deg)
    r[~np.isfinite(r)] = 0.0
    val_e = (r[col] * r[row]).astype(np.float32)

    # per-core edge lists sorted by local dest
    cores = []
    counts = np.zeros((n_cores, NB, n_half), dtype=np.int64)
    for c in range(n_cores):
        m = (col >= c * NPC) & (col < (c + 1) * NPC)
        rc = row[m]
        cc = col[m] - c * NPC
        vc = val_e[m]
        o = np.argsort(cc, kind="stable")
        rc, cc, vc = rc[o], cc[o], vc[o]
        blk = cc >> 7
        half = (rc >= HALF).astype(np.int64)
        # bucket edges per (block, half), preserving order
        key = blk * n_half + half
        ob = np.argsort(key, kind="stable")
        rc, cc, vc, blk, half = rc[ob], cc[ob], vc[ob], blk[ob], half[ob]
        np.add.at(counts[c], (blk, half), 1)
        cores.append((rc, cc, vc, blk, half))

    # static structure: groups per (block, half) = max over cores
    G = np.maximum.reduce([np.ceil(counts[c] / 128.0).astype(np.int64)
                           for c in range(n_cores)])
    for b in range(NB):
        if G[b].sum() == 0:
            G[b, 0] = 1  # guarantee every block's psum region is written

    # Token stream AND matmul emission are block-major (for b: for h:
    # groups): each (block, half) span is one gather run (single source
    # half), each block's PSUM accumulation group is contiguous, and msg
    # tiles are consumed in stream order (bounded pool pressure).
    sched = []          # per sb: [(token_gidx, b_loc, start, stop), ...]
    sb_runs = []        # per sb: [(h, ntokens), ...] gather runs in order
    for s in range(NSB):
        blocks = list(range(s * SB_BLOCKS, s * SB_BLOCKS + sb_nblocks[s]))
        groups = []
        runs = []
        for bi, b in enumerate(blocks):
            # alternate the half order per block so adjacent blocks' runs
            # share a source half and merge into one gather run below
            order = ((0, 1) if (bi % 2 == 0 or n_half == 1) else (1, 0))[:n_half]
            metas = []
            for h in order:
                if G[b, h] > 0:
                    if runs and runs[-1][0] == h:
                        runs[-1][1] += int(G[b, h]) * 128
                    else:
                        runs.append([h, int(G[b, h]) * 128])
                for _ in range(G[b, h]):
                    metas.append([len(groups) + len(metas),
                                  b - s * SB_BLOCKS, False, False])
            metas[0][2] = True
            metas[-1][3] = True
            groups.extend(metas)
        sched.append(groups)
        sb_runs.append(runs)

    G_total = sum(len(g) for g in sched)

    # per-core token arrays in schedule order
    # numpy has no native bfloat16; round via ml_dtypes (shipped with jax)
    import ml_dtypes
    in_maps = []
    xbf = np.zeros((N + 128, F), dtype=ml_dtypes.bfloat16)
    xbf[:N] = x.astype(ml_dtypes.bfloat16)
    xpad = np.zeros((N + 128, F), dtype=np.float32)
    xpad[:N] = x
    epad = np.zeros((N + 128, K), dtype=np.float32)
    epad[:N] = e
    WF = np.ascontiguousarray(
        weights.astype(np.float32).transpose(1, 0, 2).reshape(2 * F, K * F))
    W_dram = np.concatenate([WF[:F], WF[F:]], axis=1)  # [128, 2*K*F]
    W_dram = W_dram.astype(ml_dtypes.bfloat16)

    for c in range(n_cores):
        rc, cc, vc, blk, half = cores[c]
        idx_parts, colrel_parts, val_parts = [], [], []
        for s in range(NSB):
            blocks = range(s * SB_BLOCKS, s * SB_BLOCKS + sb_nblocks[s])
            for bi, b in enumerate(blocks):
                for h in ((0, 1) if (bi % 2 == 0 or n_half == 1) else (1, 0))[:n_half]:
                    m = (blk == b) & (half == h)
                    ridx = rc[m] - h * HALF
                    crel = (cc[m] - b * 128).astype(np.float32)
                    v = vc[m]
                    if _os.environ.get("KERNEL_IDX_ZERO"):
                        # timing probe: all tokens from one hot 32KB region
                        ridx = ridx % 128
                    if _os.environ.get("KERNEL_SORT_SRC", "1") != "0":
                        # ascending source addresses within the bucket:
                        # better HBM row-buffer locality for the gather
                        oo = np.argsort(ridx, kind="stable")
                        ridx, crel, v = ridx[oo], crel[oo], v[oo]
                    n = m.sum()
                    npad = G[b, h] * 128 - n
                    assert npad >= 0
                    idx_parts.append(np.concatenate(
                        [ridx, np.zeros(npad, np.int64)]).astype(np.int16))
                    colrel_parts.append(np.concatenate(
                        [crel, np.full(npad, PAD_COLREL, np.float32)]))
                    val_parts.append(np.concatenate(
                        [v, np.zeros(npad, np.float32)]))
        idx_cat = np.concatenate(idx_parts)
        colrel_cat = np.concatenate(colrel_parts)
        val_cat = np.concatenate(val_parts)
        assert idx_cat.shape[0] == G_total * 128

        # wrapped idx stream, sliced per (sb, block, half) gather sub-run
        idx_w = []
        off = 0
        for s in range(NSB):
            for _h, ntok in sb_runs[s]:
                rem = ntok
                while rem > 0:
                    take = min(SUBRUN, rem)
                    idx_w.append(_wrap16(idx_cat[off:off + take]))
                    off += take
                    rem -= take
        idx_dram = (np.concatenate(idx_w, axis=1) if idx_w
                    else np.zeros((128, 16), np.int16))

        colrel_dram = np.ascontiguousarray(
            colrel_cat.reshape(G_total, 128).T).astype(ml_dtypes.bfloat16)
        val_dram = np.ascontiguousarray(
            val_cat.reshape(G_total, 128).T).astype(ml_dtypes.bfloat16)
        s_dram = None
        if _os.environ.get("KERNEL_S_MODE", "dram") == "dram":
            # dense static S: S[p, g*128+c] = (colrel[g*128+p]==c)*val[...]
            crm = colrel_dram.astype(np.int16)            # [128, G]
            valm = val_dram                               # [128, G] bf16
            oneh = (crm[:, :, None] ==
                    np.arange(128, dtype=np.int16)[None, None, :])
            s_dram = (oneh * valm[:, :, None].astype(np.float32)).astype(
                ml_dtypes.float8_e4m3).reshape(128, G_total * 128)
            s_dram = np.ascontiguousarray(s_dram)

        xT = np.ascontiguousarray(
            xbf[c * NPC:c * NPC + NPCP].T)
        x_res = np.ascontiguousarray(xpad[c * NPC:c * NPC + NPCP])
        e_gate = np.ascontiguousarray(
            epad[c * NPC:c * NPC + NPCP].reshape(NB, 128, K)
            .transpose(1, 0, 2).reshape(128, NB * K))

        in_maps.append({
            "x_full": np.ascontiguousarray(xbf),
            "xT": xT,
            "x_res": x_res,
            "e_gate": e_gate,
            "W": np.ascontiguousarray(W_dram),
            "iota_w": np.tile(np.arange(128, dtype=np.float32),
                              (128, 1)).astype(ml_dtypes.bfloat16),
            "idx": np.ascontiguousarray(idx_dram),
            "colrel": colrel_dram,
            "val": val_dram,
            "chain": np.zeros((1, 128), np.float32),
        })
        if s_dram is not None:
            in_maps[-1]["S"] = s_dram
        import os as _os1
        _v = _os1.environ.get("KERNEL_VARIANT", "full")
        if _v == "nogather":
            in_maps[-1]["fake_msg"] = np.zeros((128, SUBRUN),
                                               ml_dtypes.bfloat16)
        elif _v.endswith("2x"):
            in_maps[-1]["x_full2"] = np.zeros(((N + 128) // 2, 2 * F),
                                              ml_dtypes.bfloat16)

    meta = dict(N=N, F=F, K=K, E=E, NPC=NPC, NB=NB, NPCP=NPCP, NSB=NSB,
                sb_nblocks=sb_nblocks, n_half=n_half, HALF=HALF,
                sched=sched, sb_runs=sb_runs, G_total=G_total,
                idx_cols=in_maps[0]["idx"].shape[1],
                n_cores=n_cores)
    return meta, in_maps


def _build(meta, reps=1):
    """Trace the Bass/Tile kernel for the static structure in meta.

    reps > 1 unrolls the whole kernel body (including its input DMA
    loads) multiple times inside one NEFF, with SBUF/PSUM pools
    recycled per repetition. A (t(reps=R) - t(reps=1)) / (R-1) slope
    between two NEFFs then measures true per-execution device time,
    cancelling the fixed per-dispatch overhead of the axon path.
    """
    from contextlib import ExitStack

    N, F, K = meta["N"], meta["F"], meta["K"]
    NB, NPCP, NSB = meta["NB"], meta["NPCP"], meta["NSB"]
    sb_nblocks, n_half, HALF = meta["sb_nblocks"], meta["n_half"], meta["HALF"]
    sched, sb_runs = meta["sched"], meta["sb_runs"]

    nc = bacc.Bacc("TRN2", target_bir_lowering=False, debug=False,
                   num_devices=meta["n_cores"], num_swdge_queues=4)

    x_full = nc.dram_tensor("x_full", [N + 128, F], BF16, kind="ExternalInput")
    xT_d = nc.dram_tensor("xT", [128, NPCP], BF16, kind="ExternalInput")
    x_res_d = nc.dram_tensor("x_res", [NPCP, F], F32, kind="ExternalInput")
    e_gate_d = nc.dram_tensor("e_gate", [128, NB * K], F32,
                              kind="ExternalInput")
    KF = K * F
    W_d = nc.dram_tensor("W", [128, 2 * KF], BF16, kind="ExternalInput")
    iota_d = nc.dram_tensor("iota_w", [128, 128], BF16, kind="ExternalInput")
    idx_d = nc.dram_tensor("idx", [128, meta["idx_cols"]], I16,
                           kind="ExternalInput")
    colrel_d = nc.dram_tensor("colrel", [128, meta["G_total"]], BF16,
                              kind="ExternalInput")
    val_d = nc.dram_tensor("val", [128, meta["G_total"]], BF16,
                           kind="ExternalInput")
    out_d = nc.dram_tensor("out", [NPCP, F], F32, kind="ExternalOutput")
    import os as _os0
    variant = _os0.environ.get("KERNEL_VARIANT", "full")
    s_mode = _os0.environ.get("KERNEL_S_MODE", "dram")
    S_d = (nc.dram_tensor("S", [128, meta["G_total"] * 128], FP8,
                          kind="ExternalInput")
           if s_mode == "dram" else None)
    fake_d = (nc.dram_tensor("fake_msg", [128, SUBRUN], BF16,
                             kind="ExternalInput")
              if variant == "nogather" else None)
    x2_d = (nc.dram_tensor("x_full2", [(N + 128) // 2, 2 * F], BF16,
                           kind="ExternalInput")
            if variant.endswith("2x") else None)
    # tiny chain tensors so a benchmark can sequence K executions of this
    # NEFF inside one jit call (data dependency defeats CSE/DCE)
    chain_i = nc.dram_tensor("chain", [1, 128], F32, kind="ExternalInput")
    chain_o = nc.dram_tensor("chain_out", [1, 128], F32,
                             kind="ExternalOutput")

    W_COLS = 2 * KF
    assert KF == 1024 and F == 128, "einsum slicing hardcoded for K=8, F=128"

    # idx cols consumed by each superblock's gathers (for chunked loads)
    sb_idx_cols = []
    for s in range(NSB):
        cols = 0
        for _h, ntok in sb_runs[s]:
            rem = ntok
            while rem > 0:
                take = min(SUBRUN, rem)
                cols += take // 16
                rem -= take
        sb_idx_cols.append(cols)

    import os as _os
    skip_einsum = bool(_os.environ.get("KERNEL_SKIP_EINSUM"))

    def _setap(ap_obj, dims):
        ap_obj.ap = mybir.VecI64Pair(dims)
        return ap_obj

    def _bcast128(t_ap):
        """[128, G] AP -> [128, G, 128] with 0-stride innermost dim."""
        ap = t_ap.copy()
        return _setap(ap, [list(d) for d in ap.ap] + [[0, 128]])

    def _as3d(t_ap, G):
        """[128, G*128] AP -> [128, G, 128]."""
        ap = t_ap.copy()
        return _setap(ap, [list(ap.ap[0]), [128, G], [1, 128]])

    def _emit_body(tc, ctx, rep):
        const = ctx.enter_context(tc.tile_pool(name=f"const{rep}", bufs=1))
        msgp = ctx.enter_context(tc.tile_pool(
            name=f"msg{rep}", bufs=int(_os.environ.get("KERNEL_MSG_BUFS", "16"))))
        import os as _osp
        sp = ctx.enter_context(tc.tile_pool(
            name=f"sp{rep}", bufs=int(_osp.environ.get("KERNEL_SW_BUFS", "3"))))
        hiTp = ctx.enter_context(tc.tile_pool(name=f"hiT{rep}", bufs=NSB))
        accp = ctx.enter_context(tc.tile_pool(name=f"acc{rep}", bufs=6))
        psag = ctx.enter_context(tc.tile_pool(name=f"psag{rep}", bufs=3,
                                              space="PSUM"))
        psmm = ctx.enter_context(tc.tile_pool(
            name=f"psmm{rep}", bufs=int(_os.environ.get("KERNEL_PSMM_BUFS", "4")),
            space="PSUM"))

        # persistent tiles; idx is loaded in per-superblock chunks (first
        # chunk first) so early gathers don't wait on the full idx stream
        # or the other constant loads.
        idx_t = const.tile([128, meta["idx_cols"]], I16, tag="idx")
        iota_t = const.tile([128, 128], BF16, tag="iota")
        cr_t = const.tile([128, meta["G_total"]], BF16, tag="cr")
        val_t = const.tile([128, meta["G_total"]], BF16, tag="val")
        w_t = const.tile([128, W_COLS], BF16, tag="w")
        xT_t = const.tile([128, NPCP], BF16, tag="xT")
        eg_t = const.tile([128, NB * K], F32, tag="eg")
        off = 0
        for s in range(NSB):
            if sb_idx_cols[s]:
                nc.sync.dma_start(idx_t[:, off:off + sb_idx_cols[s]],
                                  idx_d.ap()[:, off:off + sb_idx_cols[s]])
                off += sb_idx_cols[s]
            if s == 0:
                nc.sync.dma_start(iota_t[:], iota_d.ap()[:, :])
                nc.sync.dma_start(cr_t[:], colrel_d.ap()[:, :])
                nc.sync.dma_start(val_t[:], val_d.ap()[:, :])
                nc.sync.dma_start(w_t[:], W_d.ap()[:, :])
                nc.sync.dma_start(xT_t[:], xT_d.ap()[:, :])
                nc.sync.dma_start(eg_t[:], e_gate_d.ap()[:, :])

        x_half = [x_full.ap()[h * HALF:N + 128, :] for h in range(n_half)]

        sconst_t = None
        if variant in ("noS", "sdecouple"):
            sconst_t = const.tile([128, 128], BF16, tag="sconst")
            nc.vector.tensor_copy(sconst_t[:], iota_t[:, :128])

        g_base = 0
        idx_off = 0
        n_gathers = 0
        for s in range(NSB):
            nb = sb_nblocks[s]
            groups = sched[s]
            ps_hi = psag.tile([128, nb * 128], F32, tag="psag")

            # gather msg tiles: one run per (block, half), in SUBRUN chunks
            tok_map = []            # token-order group idx -> (tile, slot)
            for h, ntok in sb_runs[s]:
                rem = ntok
                while rem > 0:
                    take = min(SUBRUN, rem)
                    if variant == "gatherhalf":
                        # timing probe: half descriptors AND half bytes
                        # (gather fills only the first half of each tile)
                        take2 = max(128, -(-take // 256) * 128)
                        mt = msgp.tile([128, take // 128, 128], BF16,
                                       tag="msg")
                        nc.gpsimd.dma_gather(
                            mt[:, :take2 // 128, :], x_half[h],
                            idx_t[:, idx_off:idx_off + take2 // 16],
                            take2, take2, F, queue_num=n_gathers % 4)
                    elif x2_d is not None:
                        # timing probe: same bytes, half the descriptors
                        # (512B tokens of 2F elems from a fake 2F-wide x)
                        take2 = max(128, -(-take // 256) * 128)
                        mt = msgp.tile([128, take2 // 128, 2 * F], BF16,
                                       tag="msg")
                        nc.gpsimd.dma_gather(
                            mt[:], x2_d.ap()[:, :],
                            idx_t[:, idx_off:idx_off + take2 // 16],
                            take2, take2, 2 * F, queue_num=n_gathers % 4)
                    else:
                        mt = msgp.tile([128, take // 128, 128], BF16,
                                       tag="msg")
                        if fake_d is not None:
                            nc.sync.dma_start(mt[:], fake_d.ap()[:, :take])
                        else:
                            import os as _osq
                            nc.gpsimd.dma_gather(
                                mt[:], x_half[h],
                                idx_t[:, idx_off:idx_off + take // 16],
                                take, take, F, queue_num=n_gathers % 4,
                                single_packet=_osq.environ.get(
                                    "KERNEL_SINGLE_PACKET", "0") != "0")
                    n_gathers += 1
                    if variant == "gatherhalf":
                        for j in range(take // 128):
                            tok_map.append((mt, j, 0, False))
                    elif x2_d is not None:
                        for j in range(take // 128):
                            tok_map.append((mt, j, (j % 2) * 128, True))
                    else:
                        for j in range(take // 128):
                            tok_map.append((mt, j, 0, False))
                    idx_off += take // 16
                    rem -= take

            # S build (2 wide DVE ops per superblock: S[:, g*128+c] =
            # (iota==colrel[g]) * val[g], broadcast along c) + aggregation
            # matmuls in block-major emission order
            if variant.startswith("gatheronly"):
                continue
            Gs = len(groups)
            if variant not in ("noS",):
                sw_t = sp.tile([128, Gs * 128], FP8 if S_d is not None
                               else BF16, tag="sw")
                if S_d is not None:
                    nc.sync.dma_start(
                        sw_t[:],
                        S_d.ap()[:, g_base * 128:(g_base + Gs) * 128])
                else:
                    iota_b = _setap(iota_t[:].copy(),
                                    [list(iota_t[:].ap[0]), [0, Gs], [1, 128]])
                    nc.vector.tensor_tensor(
                        _as3d(sw_t[:], Gs), iota_b,
                        _bcast128(cr_t[:, g_base:g_base + Gs]),
                        mybir.AluOpType.is_equal)
                    nc.vector.tensor_tensor(
                        _as3d(sw_t[:], Gs), _as3d(sw_t[:], Gs),
                        _bcast128(val_t[:, g_base:g_base + Gs]),
                        mybir.AluOpType.mult)
            for tg, b_loc, start, stop in groups:
                s_t = (sconst_t if variant in ("noS", "sdecouple")
                       else sw_t[:, tg * 128:(tg + 1) * 128])
                if variant == "noPE":
                    continue
                mt, j, coff, wide = tok_map[tg]
                msrc = (mt[:, j // 2:j // 2 + 1, coff:coff + 128] if wide
                        else mt[:, j:j + 1, :])
                nc.tensor.matmul(
                    ps_hi[:, b_loc * 128:(b_loc + 1) * 128],
                    msrc, s_t, start=start, stop=stop)
            g_base += len(groups)

            if variant == "noPE":
                continue
            hiT_t = hiTp.tile([128, nb * 128], BF16, tag="hiT")
            nc.vector.tensor_copy(hiT_t[:], ps_hi[:])

            if skip_einsum:
                for b_loc in range(nb):
                    b = s * SB_BLOCKS + b_loc
                    acc = accp.tile([128, F], F32, tag="acc")
                    nc.vector.tensor_copy(acc[:],
                                          hiT_t[:, b_loc * 128:(b_loc + 1) * 128])
                    nc.sync.dma_start(out_d.ap()[b * 128:(b + 1) * 128, :],
                                      acc[:])
                continue

            # einsum + gating + residual per block
            for b_loc in range(nb):
                b = s * SB_BLOCKS + b_loc
                hiT_b = hiT_t[:, b_loc * 128:(b_loc + 1) * 128]
                xT_b = xT_t[:, b * 128:(b + 1) * 128]
                pa = psmm.tile([128, 512], F32, tag="pmm")
                pb = psmm.tile([128, 512], F32, tag="pmm")
                nc.tensor.matmul(pa[:], hiT_b, w_t[:, 0:512],
                                 start=True, stop=False)
                nc.tensor.matmul(pb[:], hiT_b, w_t[:, 512:1024],
                                 start=True, stop=False)
                nc.tensor.matmul(pa[:], xT_b, w_t[:, 1024:1536],
                                 start=False, stop=True)
                nc.tensor.matmul(pb[:], xT_b, w_t[:, 1536:2048],
                                 start=False, stop=True)
                acc = accp.tile([128, F], F32, tag="acc")
                nc.sync.dma_start(acc[:], x_res_d.ap()[b * 128:(b + 1) * 128, :])
                for k in range(K):
                    src = pa if k < 4 else pb
                    kk = k % 4
                    nc.vector.scalar_tensor_tensor(
                        acc[:], src[:, kk * 128:(kk + 1) * 128],
                        eg_t[:, b * K + k:b * K + k + 1], acc[:],
                        mybir.AluOpType.mult, mybir.AluOpType.add)
                nc.sync.dma_start(out_d.ap()[b * 128:(b + 1) * 128, :], acc[:])

    with tile.TileContext(nc) as tc:
        with ExitStack() as chctx:
            chp = chctx.enter_context(tc.tile_pool(name="chp", bufs=1))
            ch_t = chp.tile([1, 128], F32, tag="ch")
            nc.sync.dma_start(ch_t[:], chain_i.ap()[:, :])
            nc.sync.dma_start(chain_o.ap()[:, :], ch_t[:])
        for rep in range(reps):
            with ExitStack() as ctx:
                _emit_body(tc, ctx, rep)

    nc.compile()
    return nc


def _bench_setup(nc, in_maps, n_cores):
    """Stage inputs and jit one execution of nc's NEFF.

    Returns (timer, results_fn): timer(depth) dispatches `depth`
    asynchronous calls, blocks, and returns wall/depth (pipelined
    throughput — amortizes the dispatch latency of the axon tunnel);
    results_fn() runs once and returns per-core output dicts."""
    import time

    import jax
    from jax.sharding import Mesh, PartitionSpec
    from jax.experimental.shard_map import shard_map

    from concourse import bass2jax, mybir as _mb
    from concourse.bass2jax import _bass_exec_p, partition_id_tensor

    bass2jax.install_neuronx_cc_hook()

    partition_name = (nc.partition_id_tensor.name
                      if nc.partition_id_tensor else None)
    in_names, out_names, out_avals, zero_outs = [], [], [], []
    for alloc in nc.m.functions[0].allocations:
        if not isinstance(alloc, _mb.MemoryLocationSet):
            continue
        name = alloc.memorylocations[0].name
        if alloc.kind == "ExternalInput":
            if name != partition_name:
                in_names.append(name)
        elif alloc.kind == "ExternalOutput":
            shape = tuple(alloc.tensor_shape)
            dtype = _mb.dt.np(alloc.dtype)
            out_names.append(name)
            out_avals.append(jax.core.ShapedArray(shape, dtype))
            zero_outs.append(np.zeros(shape, dtype))
    n_params = len(in_names)
    all_in_names = in_names + out_names
    if partition_name is not None:
        all_in_names = all_in_names + [partition_name]

    def _body(*args):
        operands = list(args)
        if partition_name is not None:
            operands.append(partition_id_tensor())
        return tuple(_bass_exec_p.bind(
            *operands, out_avals=tuple(out_avals),
            in_names=tuple(all_in_names), out_names=tuple(out_names),
            lowering_input_output_aliases=(), sim_require_finite=True,
            sim_require_nnan=True, nc=nc))

    devices = jax.devices()[:n_cores]
    mesh = Mesh(np.asarray(devices), ("core",))
    nin = n_params + len(out_names)
    sh = jax.sharding.NamedSharding(mesh, PartitionSpec("core"))
    concat_in = [jax.device_put(
        np.concatenate([np.asarray(in_maps[c][k]) for c in range(n_cores)], 0),
        sh) for k in in_names]
    concat_zeros = [jax.device_put(
        np.zeros((n_cores * z.shape[0], *z.shape[1:]), z.dtype), sh)
        for z in zero_outs]
    fn = jax.jit(shard_map(_body, mesh=mesh,
                           in_specs=(PartitionSpec("core"),) * nin,
                           out_specs=(PartitionSpec("core"),) * len(out_names),
                           check_rep=False), keep_unused=True)
    out = fn(*concat_in, *concat_zeros)   # warmup (compile+load)
    jax.block_until_ready(out)

    def timer(depth):
        t0 = time.perf_counter()
        o = None
        for _ in range(depth):
            o = fn(*concat_in, *concat_zeros)
        jax.block_until_ready(o)
        return (time.perf_counter() - t0) / depth

    def results_fn():
        return [{name: np.asarray(out[i]).reshape(
                    n_cores, *out_avals[i].shape)[c]
                 for i, name in enumerate(out_names)} for c in range(n_cores)]
    return timer, results_fn


BENCH_REPS_LO = 5      # bodies in the two NEFFs of the slope measurement
BENCH_REPS_HI = 21


def _run(x, adj, e, weights, n_cores=8, sim=False, trace=False):
    meta, in_maps = _prep(x, adj, e, weights, n_cores)
    nc = _build(meta)
    N, F, NPC, NPCP = meta["N"], meta["F"], meta["NPC"], meta["NPCP"]

    if sim:
        from concourse.bass_interp import CoreSim
        outs = []
        for c in range(n_cores):
            simr = CoreSim(nc)
            for k, v in in_maps[c].items():
                simr.tensor(k)[:] = v
            simr.simulate(check_with_hw=False)
            outs.append(np.array(simr.tensor("out")))
        res = None
    elif trace:
        # Per-execution device time via the two-NEFF slope: the same body
        # unrolled BENCH_REPS_LO x and BENCH_REPS_HI x inside one NEFF,
        # timed with 16-deep pipelined dispatch. The fixed per-dispatch
        # overhead of the axon tunnel (~6-7 ms/call pipelined, ~85 ms
        # round-trip, measured with an empty NEFF) is identical for both
        # body counts, so it cancels out of the slope.
        timer1, results_fn = _bench_setup(nc, in_maps, n_cores)
        timer1(1)
        results = results_fn()
        nc_lo = _build(meta, reps=BENCH_REPS_LO)
        nc_hi = _build(meta, reps=BENCH_REPS_HI)
        timer_lo, _ = _bench_setup(nc_lo, in_maps, n_cores)
        timer_hi, _ = _bench_setup(nc_hi, in_maps, n_cores)
        # interleave lo/hi reps so tunnel drift cancels out of the slope
        t_lo, t_hi = float("inf"), float("inf")
        for _ in range(6):
            t_lo = min(t_lo, timer_lo(32))
            t_hi = min(t_hi, timer_hi(32))
        per_iter_ns = (t_hi - t_lo) / (BENCH_REPS_HI - BENCH_REPS_LO) * 1e9
        print(f"  [bench] pipelined wall reps={BENCH_REPS_LO}: "
              f"{t_lo*1e3:.2f} ms, reps={BENCH_REPS_HI}: {t_hi*1e3:.2f} ms",
              flush=True)
        outs = [r["out"] for r in results]
        res = per_iter_ns
    else:
        from concourse.bass_utils import run_bass_kernel_spmd
        res = run_bass_kernel_spmd(nc, in_maps, core_ids=list(range(n_cores)),
                                   trace=trace)
        outs = [r["out"] for r in res.results]

    out = np.concatenate([o[:NPC] for o in outs], axis=0)
    assert out.shape == (N, F)
    return out.astype(np.float32), res


def kernel(x, adj, e, weights):
    x = np.asarray(x, dtype=np.float32)
    adj = np.asarray(adj)
    e = np.asarray(e, dtype=np.float32)
    weights = np.asarray(weights, dtype=np.float32)
    out, _ = _run(x, adj, e, weights, n_cores=8, sim=False)
    return out
